# revision 20
# baseline (speedup 1.0000x reference)
"""Trainium2 Bass kernel for the AGSG/MHSG graph-attention problem.

Computes, for x [16,64,512,12] and memory [64,512] (both f32):
  A_p = softmax(relu(x_sum[:, :, None] * sup_sum[None] / 8), -1)   [16,512,512]
  A_l = softmax(relu(gram(xws) / 8), -1)                            [16,512,512]
where sup_sum = sum_{k=0..512} S_w^k and S_w = softmax(relu(mem.T@mem) w/ diag 0.1).

Key algebraic facts used (all verified numerically against the reference):
  * S_w is a dense positive stochastic matrix with |lambda_2| ~ 5e-3, so
    S_w^k converges to 1*pi^T almost immediately:
        sup_sum = I + S_w + 511 * 1 pi^T   (error ~2e-6)
    with pi obtained by two power iterations from the uniform vector.
  * rowsum(S_w) == 1 to fp precision, so the supra-Laplacian row-sum vector
    rs is the compile-time constant 1 + 0.8*(11 - i//512) (chunk-constant
    along the flattened (n,t) axis) -> folded into per-chunk ACT exp scales.
  * relu inside A_p's softmax reduces to clamping the per-row scalar
    x_sum/8 at 0 (sup_sum > 0 elementwise); relu before A_l's softmax is a
    no-op (gram > 0); no softmax needs max-subtraction (|logit| <= ~34).

Distribution: pure data-parallel, batch 16 -> 8 cores x 2. memory is
replicated; the tiny S-chain is recomputed on every core.
"""

import numpy as np

import concourse.bass as bass
import concourse.bacc as bacc
import concourse.tile as tile
from concourse import mybir
from concourse.bass_utils import run_bass_kernel_spmd

F32 = mybir.dt.float32
BF16 = mybir.dt.bfloat16
AF = mybir.ActivationFunctionType
OP = mybir.AluOpType
AX = mybir.AxisListType

# Problem constants (hardcoded per harness contract).
B, C, N, T = 16, 64, 512, 12
ALPH = 0.8
ISC = 0.125          # 1/sqrt(C)
NCORES = 8
BPC = B // NCORES    # batches per core = 2
P = 128              # SBUF partitions
NTILE = N // P       # 4 row tiles of the NxN outputs
NT = N * T           # 6144
NCH = 4              # x processing chunks
CHF = NT // NCH      # 1536 free elems per chunk
PI_ITERS = 2


def _body(ctx, nc, tc, x_d, mem_d, eye_d, out_d):
    constp = ctx.enter_context(tc.tile_pool(name="const", bufs=1))
    xinp = ctx.enter_context(tc.tile_pool(name="xin", bufs=1))
    sp = ctx.enter_context(tc.tile_pool(name="schain", bufs=1))
    smallp = ctx.enter_context(tc.tile_pool(name="small", bufs=1))
    stagep = ctx.enter_context(tc.tile_pool(name="stage", bufs=2))
    psA = ctx.enter_context(tc.tile_pool(name="psA", bufs=1, space="PSUM"))
    psB = ctx.enter_context(tc.tile_pool(name="psB", bufs=2, space="PSUM"))
    psS = ctx.enter_context(tc.tile_pool(name="psS", bufs=1, space="PSUM"))

    x_flat = x_d[:].rearrange("b c n t -> (b c) (n t)")
    out_v = out_d[:].rearrange("b o (t p) m -> b o p t m", p=P)

    # ---------------- input DMAs + constants ----------------
    m_sb = sp.tile([C, N], F32)
    nc.sync.dma_start(m_sb[:], mem_d[:])
    eye = constp.tile([P, P], F32)
    nc.sync.dma_start(eye[:], eye_d[:])
    x_sb = xinp.tile([P, NT], F32)
    for j in range(NCH):
        nc.sync.dma_start(x_sb[:, j * CHF:(j + 1) * CHF],
                          x_flat[:, j * CHF:(j + 1) * CHF])

    ones64 = constp.tile([C, 1], BF16)
    nc.vector.memset(ones64[:], 1.0)
    c511 = constp.tile([1, P], F32)
    nc.vector.memset(c511[:], 511.0)
    bones = constp.tile([P, BPC], F32)
    nc.vector.memset(bones[:], 0.0)
    for b in range(BPC):
        nc.vector.memset(bones[b * C:(b + 1) * C, b:b + 1], ISC)
    eye_bf = constp.tile([P, P], BF16)
    nc.vector.tensor_copy(eye_bf[:], eye[:])
    m_bf = sp.tile([C, N], BF16)
    nc.vector.tensor_copy(m_bf[:], m_sb[:])

    # ---------------- S chain (small, overlaps the x pipeline) ----------
    # s0 = m^T m (bf16 PE), diag := 0.1 via accumulated diag matmul
    s0_ps = psA.tile([P, NTILE, N], F32, tag="big")
    for t in range(NTILE):
        nc.tensor.matmul(s0_ps[:, t, :], lhsT=m_bf[:, t * P:(t + 1) * P],
                         rhs=m_bf[:], start=True, stop=False,
                         skip_group_check=True)
    msq = sp.tile([C, N], BF16)
    nc.scalar.activation(msq[:], m_bf[:], AF.Square)
    dc_ps = psS.tile([P, NTILE], F32, tag="colp")
    for t in range(NTILE):
        nc.tensor.matmul(dc_ps[:, t:t + 1], lhsT=msq[:, t * P:(t + 1) * P],
                         rhs=ones64[:], start=True, stop=True,
                         skip_group_check=True)
    d_col = smallp.tile([P, NTILE], F32, tag="dcol")
    nc.vector.tensor_scalar(d_col[:], dc_ps[:], -1.0, 0.1, OP.mult, OP.add)
    for t in range(NTILE):
        dg = smallp.tile([P, P], BF16, tag="diag")
        nc.vector.tensor_scalar(dg[:], eye_bf[:], d_col[:, t:t + 1], None, OP.mult)
        nc.tensor.matmul(s0_ps[:, t, t * P:(t + 1) * P], lhsT=dg[:], rhs=eye_bf[:],
                         start=False, stop=True, skip_group_check=True)

    # relu(s0) in place on PSUM (ACT), then E = exp(relu(s0)) >= 1 with fused
    # row-sum accums (z).  No max(.,1) needed since relu'd logits are >= 0.
    nc.scalar.activation(s0_ps[:], s0_ps[:], AF.Relu)
    E_all = sp.tile([P, NTILE, N], BF16)
    zc = smallp.tile([P, NTILE], F32, tag="zc")
    for t in range(NTILE):
        nc.scalar.activation(E_all[:, t, :], s0_ps[:, t, :], AF.Exp,
                             accum_out=zc[:, t:t + 1])

    # ---------------- x pipeline (DVE-bound; S-chain small ops weave in) ----
    xr = xinp.tile([P, NT], F32)
    s12 = sp.tile([P, N], F32)
    xt = sp.tile([P, N], F32)
    x3 = x_sb[:].rearrange("p (n t) -> p n t", t=T)
    xr3 = xr[:].rearrange("p (n t) -> p n t", t=T)
    NW = N // NCH  # n's per chunk

    def chunk_front(j):
        # relu(ck * x) per 512-wide constant-rs chunk; 2 on ACT, 1 on DVE
        for k in range(3 * j, 3 * j + 3):
            ck = (1.0 + ALPH * (T - 1 - k)) * ISC
            if k % 3 == 2:
                nc.vector.tensor_scalar(xr[:, k * N:(k + 1) * N],
                                        x_sb[:, k * N:(k + 1) * N],
                                        0.0, ck, OP.max, OP.mult)
            else:
                nc.scalar.activation(xr[:, k * N:(k + 1) * N],
                                     x_sb[:, k * N:(k + 1) * N],
                                     AF.Relu, scale=ck)
        nc.scalar.activation(xr[:, j * CHF:(j + 1) * CHF],
                             xr[:, j * CHF:(j + 1) * CHF], AF.Exp)
        nc.vector.reduce_sum(xt[:, j * NW:(j + 1) * NW],
                             x3[:, j * NW:(j + 1) * NW, :], axis=AX.X)
        nc.vector.reduce_sum(s12[:, j * NW:(j + 1) * NW],
                             xr3[:, j * NW:(j + 1) * NW, :], axis=AX.X)

    chunk_front(0)
    chunk_front(1)

    # S-chain tail: r = 1/z, pi ~= (r/N)^T E, sup = 511*1 pi^T + diag(r) E + I
    r_col = smallp.tile([P, NTILE], F32, tag="rcol")
    nc.vector.reciprocal(r_col[:], zc[:])
    u = smallp.tile([P, NTILE], BF16, tag="u0")
    nc.vector.tensor_scalar(u[:], r_col[:], 1.0 / N, None, OP.mult)
    v_ps = psS.tile([1, N], F32, tag="rowp")
    for kt in range(NTILE):
        nc.tensor.matmul(v_ps[:], lhsT=u[:, kt:kt + 1], rhs=E_all[:, kt, :],
                         start=(kt == 0), stop=(kt == NTILE - 1))
    pirow = smallp.tile([1, N], F32, tag="vrow")
    nc.vector.tensor_copy(pirow[:], v_ps[:])
    drgs = smallp.tile([P, NTILE, P], BF16, tag="drgs")
    for t in range(NTILE):
        nc.vector.tensor_scalar(drgs[:, t, :], eye_bf[:], r_col[:, t:t + 1],
                                None, OP.mult)
    sup_ps = psA.tile([P, NTILE, N], F32, tag="big")
    for t in range(NTILE):
        nc.tensor.matmul(sup_ps[:, t, :], lhsT=c511[:], rhs=pirow[:],
                         start=True, stop=False, skip_group_check=True)
        nc.tensor.matmul(sup_ps[:, t, :], lhsT=drgs[:, t, :], rhs=E_all[:, t, :],
                         start=False, stop=False, skip_group_check=True)
        nc.tensor.matmul(sup_ps[:, t, t * P:(t + 1) * P], lhsT=eye_bf[:],
                         rhs=eye_bf[:], start=False, stop=True,
                         skip_group_check=True)

    chunk_front(2)
    chunk_front(3)

    # xws = s12 / Z; w1 = rowsum(xws) for the A_l denominator
    Z = smallp.tile([P, 1], F32, tag="Z")
    nc.vector.reduce_sum(Z[:], s12[:], axis=AX.X)
    rZ = smallp.tile([P, 1], F32, tag="rZ")
    nc.vector.reciprocal(rZ[:], Z[:])
    xws = sp.tile([P, N], BF16)
    nc.vector.tensor_scalar(xws[:], s12[:], rZ[:], None, OP.mult)
    w1f = smallp.tile([P, 1], F32, tag="w1f")
    nc.vector.reduce_sum(w1f[:], xws[:], axis=AX.X)
    w1 = smallp.tile([P, 1], BF16, tag="w1")
    nc.vector.tensor_copy(w1[:], w1f[:])

    # sc[n, (t,b)] = max(x_sum/8, 0) in n-on-partitions layout
    xs_ps = psS.tile([BPC, N], F32, tag="rowp")
    nc.tensor.matmul(xs_ps[:], lhsT=bones[:], rhs=xt[:], start=True, stop=True)
    xs_sb = smallp.tile([BPC, N], F32, tag="xssb")
    nc.vector.tensor_copy(xs_sb[:], xs_ps[:])
    sc_ps = psS.tile([P, NTILE * BPC], F32, tag="colp")
    for t in range(NTILE):
        nc.tensor.transpose(sc_ps[:, t * BPC:(t + 1) * BPC],
                            xs_sb[:, t * P:(t + 1) * P], eye[0:BPC, 0:BPC])
    sc_sb = smallp.tile([P, NTILE * BPC], F32, tag="scsb")
    nc.vector.tensor_scalar(sc_sb[:], sc_ps[:], 0.0, None, OP.max)

    # A_l denominator pieces: sigma[n] = sum_c xws[c,n] w1[c];
    # rl = 1/(512 + sigma/8); A_l = rl + (rl/8) * gram  (exp(u) ~= 1+u)
    sig_ps = psS.tile([P, BPC * NTILE], F32, tag="colp")
    for b in range(BPC):
        for t in range(NTILE):
            col = b * NTILE + t
            nc.tensor.matmul(sig_ps[:, col:col + 1],
                             lhsT=xws[C * b:C * (b + 1), t * P:(t + 1) * P],
                             rhs=w1[C * b:C * (b + 1), :], start=True, stop=True)
    den = smallp.tile([P, BPC * NTILE], F32, tag="den")
    nc.vector.tensor_scalar(den[:], sig_ps[:], ISC, float(N), OP.mult, OP.add)
    rl = smallp.tile([P, BPC * NTILE], F32, tag="rl")
    nc.vector.reciprocal(rl[:], den[:])
    rl8 = smallp.tile([P, BPC * NTILE], F32, tag="rl8")
    nc.vector.tensor_scalar(rl8[:], rl[:], ISC, None, OP.mult)

    # ---------------- outputs, interleaved per batch ----------------
    for b in range(BPC):
        ape = stagep.tile([P, NTILE, N], F32, tag="ape")
        apz = smallp.tile([P, NTILE], F32, tag="apz%d" % b)
        for t in range(NTILE):
            nc.scalar.activation(ape[:, t, :], sup_ps[:, t, :], AF.Exp,
                                 scale=sc_sb[:, t * BPC + b:t * BPC + b + 1],
                                 accum_out=apz[:, t:t + 1])
        apr = smallp.tile([P, NTILE], F32, tag="apr%d" % b)
        nc.vector.reciprocal(apr[:], apz[:])
        for t in range(NTILE):
            nc.vector.tensor_scalar(ape[:, t, :], ape[:, t, :],
                                    apr[:, t:t + 1], None, OP.mult)
        nc.sync.dma_start(out_v[b, 0], ape[:])

        ale = stagep.tile([P, NTILE, N], F32, tag="ale")
        for t in range(NTILE):
            col = b * NTILE + t
            g_ps = psB.tile([P, N], F32, tag="gram")
            nc.tensor.matmul(g_ps[:],
                             lhsT=xws[C * b:C * (b + 1), t * P:(t + 1) * P],
                             rhs=xws[C * b:C * (b + 1), :], start=True, stop=True)
            nc.scalar.activation(ale[:, t, :], g_ps[:], AF.Identity,
                                 bias=rl[:, col:col + 1],
                                 scale=rl8[:, col:col + 1])
        nc.sync.dma_start(out_v[b, 1], ale[:])


def build_nc():
    nc = bacc.Bacc("TRN2", target_bir_lowering=False, debug=False,
                   num_devices=NCORES)
    x_d = nc.dram_tensor("x", [BPC, C, N, T], F32, kind="ExternalInput")
    mem_d = nc.dram_tensor("memory", [C, N], F32, kind="ExternalInput")
    eye_d = nc.dram_tensor("eye", [P, P], F32, kind="ExternalInput")
    out_d = nc.dram_tensor("out", [BPC, 2, N, N], F32, kind="ExternalOutput")
    from contextlib import ExitStack
    with tile.TileContext(nc) as tc:
        with ExitStack() as ctx:
            _body(ctx, nc, tc, x_d, mem_d, eye_d, out_d)
    nc.compile()
    return nc


_NC = None


def _get_nc():
    global _NC
    if _NC is None:
        _NC = build_nc()
    return _NC


def run(x, memory, trace=False):
    nc = _get_nc()
    x = np.ascontiguousarray(np.asarray(x, dtype=np.float32))
    memory = np.ascontiguousarray(np.asarray(memory, dtype=np.float32))
    eye = np.eye(P, dtype=np.float32)
    in_maps = [
        {"x": np.ascontiguousarray(x[i * BPC:(i + 1) * BPC]),
         "memory": memory, "eye": eye}
        for i in range(NCORES)
    ]
    res = run_bass_kernel_spmd(nc, in_maps, core_ids=list(range(NCORES)),
                               trace=trace)
    full = np.concatenate([r["out"] for r in res.results], axis=0)
    return (full[:, 0], full[:, 1]), res


def kernel(x, memory):
    (a_p, a_l), _ = run(x, memory, trace=False)
    return a_p, a_l


# revision 23
# speedup vs baseline: 1.1230x; 1.1230x over previous
"""Trainium2 Bass kernel for the AGSG/MHSG graph-attention problem.

Computes, for x [16,64,512,12] and memory [64,512] (both f32):
  A_p = softmax(relu(x_sum[:, :, None] * sup_sum[None] / 8), -1)   [16,512,512]
  A_l = softmax(relu(gram(xws) / 8), -1)                            [16,512,512]
where sup_sum = sum_{k=0..512} S_w^k and S_w = softmax(relu(mem.T@mem) w/ diag 0.1).

Key algebraic facts used (all verified numerically against the reference):
  * S_w is a dense positive stochastic matrix with |lambda_2| ~ 5e-3, so
    S_w^k converges to 1*pi^T almost immediately:
        sup_sum = I + S_w + 511 * 1 pi^T   (error ~2e-6)
    with pi obtained by two power iterations from the uniform vector.
  * rowsum(S_w) == 1 to fp precision, so the supra-Laplacian row-sum vector
    rs is the compile-time constant 1 + 0.8*(11 - i//512) (chunk-constant
    along the flattened (n,t) axis) -> folded into per-chunk ACT exp scales.
  * relu inside A_p's softmax reduces to clamping the per-row scalar
    x_sum/8 at 0 (sup_sum > 0 elementwise); relu before A_l's softmax is a
    no-op (gram > 0); no softmax needs max-subtraction (|logit| <= ~34).

Distribution: pure data-parallel, batch 16 -> 8 cores x 2. memory is
replicated; the tiny S-chain is recomputed on every core.
"""

import numpy as np

import concourse.bass as bass
import concourse.bacc as bacc
import concourse.tile as tile
from concourse import mybir
from concourse.bass_utils import run_bass_kernel_spmd

F32 = mybir.dt.float32
BF16 = mybir.dt.bfloat16
AF = mybir.ActivationFunctionType
OP = mybir.AluOpType
AX = mybir.AxisListType

# Problem constants (hardcoded per harness contract).
B, C, N, T = 16, 64, 512, 12
ALPH = 0.8
ISC = 0.125          # 1/sqrt(C)
NCORES = 8
BPC = B // NCORES    # batches per core = 2
P = 128              # SBUF partitions
NTILE = N // P       # 4 row tiles of the NxN outputs
NT = N * T           # 6144
NCH = 4              # x processing chunks
CHF = NT // NCH      # 1536 free elems per chunk
PI_ITERS = 2


def _body(ctx, nc, tc, x_d, mem_d, eye_d, out_d):
    constp = ctx.enter_context(tc.tile_pool(name="const", bufs=1))
    xinp = ctx.enter_context(tc.tile_pool(name="xin", bufs=1))
    sp = ctx.enter_context(tc.tile_pool(name="schain", bufs=1))
    smallp = ctx.enter_context(tc.tile_pool(name="small", bufs=1))
    stagep = ctx.enter_context(tc.tile_pool(name="stage", bufs=2))
    psA = ctx.enter_context(tc.tile_pool(name="psA", bufs=1, space="PSUM"))
    psB = ctx.enter_context(tc.tile_pool(name="psB", bufs=2, space="PSUM"))
    psS = ctx.enter_context(tc.tile_pool(name="psS", bufs=1, space="PSUM"))

    F32R = mybir.dt.float32r
    EXP01 = 1.1051709180756477  # exp(0.1)

    x_flat = x_d[:].rearrange("b c n t -> (b c) (n t)")
    out_v = out_d[:].rearrange("b o (t p) m -> b o p t m", p=P)

    # ---------------- input DMAs (split across both HWDGE rings) ----------
    m_sb = sp.tile([C, N], F32)
    nc.sync.dma_start(m_sb[:], mem_d[:])
    eye = constp.tile([P, P], F32)
    nc.scalar.dma_start(eye[:], eye_d[:])
    x_sb = xinp.tile([P, NT], F32)
    for j in range(NCH):
        eng = nc.sync if j % 2 == 0 else nc.scalar
        eng.dma_start(x_sb[:, j * CHF:(j + 1) * CHF],
                      x_flat[:, j * CHF:(j + 1) * CHF])

    ones64 = constp.tile([C, 1], BF16)
    nc.vector.memset(ones64[:], 1.0)
    c511 = constp.tile([1, P], BF16)
    nc.vector.memset(c511[:], 511.0)
    bones = constp.tile([P, BPC], F32)
    nc.vector.memset(bones[:], 0.0)
    for b in range(BPC):
        nc.vector.memset(bones[b * C:(b + 1) * C, b:b + 1], ISC)
    eye_bf = constp.tile([P, P], BF16)
    nc.vector.tensor_copy(eye_bf[:], eye[:])
    m_bf = sp.tile([C, N], BF16)
    nc.vector.tensor_copy(m_bf[:], m_sb[:])

    # ---------------- S chain ----------------
    # s0 = m^T m with RAW diagonal (diag fixed algebraically downstream);
    # relu in place on PSUM, E = exp(relu(s0)) >= 1 with fused row sums.
    s0_ps = psA.tile([P, NTILE, N], F32, tag="big")
    for t in range(NTILE):
        nc.tensor.matmul(s0_ps[:, t, :], lhsT=m_bf[:, t * P:(t + 1) * P],
                         rhs=m_bf[:], start=True, stop=True,
                         skip_group_check=True)
    nc.scalar.activation(s0_ps[:], s0_ps[:], AF.Relu)
    E_all = sp.tile([P, NTILE, N], BF16)
    zc = smallp.tile([P, NTILE], F32, tag="zc")
    for t in range(NTILE):
        nc.scalar.activation(E_all[:, t, :], s0_ps[:, t, :], AF.Exp,
                             accum_out=zc[:, t:t + 1])

    # dc = diag(s0) = colsum(m_bf^2) in column layout; w = exp(0.1) - exp(dc)
    msq = sp.tile([C, N], BF16)
    nc.scalar.activation(msq[:], m_bf[:], AF.Square)
    dc_ps = psS.tile([P, NTILE], F32, tag="colp")
    for t in range(NTILE):
        nc.tensor.matmul(dc_ps[:, t:t + 1], lhsT=msq[:, t * P:(t + 1) * P],
                         rhs=ones64[:], start=True, stop=True,
                         skip_group_check=True)
    expdc = smallp.tile([P, NTILE], F32, tag="expdc")
    nc.scalar.activation(expdc[:], dc_ps[:], AF.Exp)
    w = smallp.tile([P, NTILE], F32, tag="w")
    nc.vector.tensor_scalar(w[:], expdc[:], -1.0, EXP01, OP.mult, OP.add)

    # r = 1 / (zc + w)  (z fixed for the raw diagonal)
    zfix = smallp.tile([P, NTILE], F32, tag="zfix")
    nc.vector.tensor_tensor(zfix[:], zc[:], w[:], OP.add)
    r_col = smallp.tile([P, NTILE], F32, tag="rcol")
    nc.vector.reciprocal(r_col[:], zfix[:])

    # pi^T ~= (r/N)^T E + diag-correction; E symmetric
    u_f = smallp.tile([P, NTILE], F32, tag="uf")
    nc.vector.tensor_scalar(u_f[:], r_col[:], 1.0 / N, None, OP.mult)
    u = smallp.tile([P, NTILE], BF16, tag="u0")
    nc.vector.tensor_copy(u[:], u_f[:])
    vcorr_f = smallp.tile([P, NTILE], F32, tag="vcf")
    nc.vector.tensor_tensor(vcorr_f[:], w[:], u_f[:], OP.mult)
    vcorr = smallp.tile([P, NTILE], BF16, tag="vc")
    nc.vector.tensor_copy(vcorr[:], vcorr_f[:])
    v_ps = psB.tile([1, N], F32, tag="gram")
    for kt in range(NTILE):
        nc.tensor.matmul(v_ps[:], lhsT=u[:, kt:kt + 1], rhs=E_all[:, kt, :],
                         start=(kt == 0), stop=False, skip_group_check=True)
    for kt in range(NTILE):
        nc.tensor.matmul(v_ps[0:1, kt * P:(kt + 1) * P],
                         lhsT=vcorr[:, kt:kt + 1], rhs=eye_bf[:],
                         start=False, stop=(kt == NTILE - 1),
                         skip_group_check=True)
    # pi row in bf16 hi/lo pieces (two bf16 matmuls recover full precision)
    pi_hi = smallp.tile([1, N], BF16, tag="pihi")
    nc.vector.tensor_copy(pi_hi[:], v_ps[:])
    pi_hif = smallp.tile([1, N], F32, tag="pihif")
    nc.vector.tensor_copy(pi_hif[:], pi_hi[:])
    pi_lo = smallp.tile([1, N], BF16, tag="pilo")
    nc.vector.scalar_tensor_tensor(pi_lo[:], v_ps[:], 1.0, pi_hif[:],
                                   OP.mult, OP.subtract)

    # sup (PSUM) = diag(r) E + diag(r*w) + I + 511 * 1 pi^T
    sfix = smallp.tile([P, NTILE], F32, tag="sfix")
    nc.vector.tensor_tensor(sfix[:], w[:], r_col[:], OP.mult)
    drgs = smallp.tile([P, NTILE, P], BF16, tag="drgs")
    srgs = smallp.tile([P, NTILE, P], BF16, tag="srgs")
    for t in range(NTILE):
        nc.vector.tensor_scalar(drgs[:, t, :], eye_bf[:], r_col[:, t:t + 1],
                                None, OP.mult)
        nc.vector.tensor_scalar(srgs[:, t, :], eye_bf[:], sfix[:, t:t + 1],
                                None, OP.mult)
    sup_ps = psA.tile([P, NTILE, N], F32, tag="big")
    for t in range(NTILE):
        dslc = sup_ps[:, t, t * P:(t + 1) * P]
        nc.tensor.matmul(sup_ps[:, t, :], lhsT=drgs[:, t, :],
                         rhs=E_all[:, t, :], start=True, stop=False,
                         skip_group_check=True)
        nc.tensor.matmul(dslc, lhsT=srgs[:, t, :], rhs=eye_bf[:],
                         start=False, stop=False, skip_group_check=True)
        nc.tensor.matmul(dslc, lhsT=eye_bf[:], rhs=eye_bf[:],
                         start=False, stop=False, skip_group_check=True)
        nc.tensor.matmul(sup_ps[:, t, :], lhsT=c511[:], rhs=pi_hi[:],
                         start=False, stop=False, skip_group_check=True)
        nc.tensor.matmul(sup_ps[:, t, :], lhsT=c511[:], rhs=pi_lo[:],
                         start=False, stop=True, skip_group_check=True)

    # ---------------- x pipeline ----------------
    xr = xinp.tile([P, NT], F32)
    s12 = sp.tile([P, N], F32)
    xt = sp.tile([P, N], F32)
    x3 = x_sb[:].rearrange("p (n t) -> p n t", t=T)
    xr3 = xr[:].rearrange("p (n t) -> p n t", t=T)
    NW = N // NCH  # n's per chunk

    for j in range(NCH):
        # xt (raw t-group sums) straight off the DMA; relu split ACT/DVE
        nc.vector.reduce_sum(xt[:, j * NW:(j + 1) * NW],
                             x3[:, j * NW:(j + 1) * NW, :], axis=AX.X)
        for k in range(3 * j, 3 * j + 3):
            ck = (1.0 + ALPH * (T - 1 - k)) * ISC
            if k % 3 == 2:
                nc.vector.tensor_scalar(xr[:, k * N:(k + 1) * N],
                                        x_sb[:, k * N:(k + 1) * N],
                                        0.0, ck, OP.max, OP.mult)
            else:
                nc.scalar.activation(xr[:, k * N:(k + 1) * N],
                                     x_sb[:, k * N:(k + 1) * N],
                                     AF.Relu, scale=ck)
        nc.scalar.activation(xr[:, j * CHF:(j + 1) * CHF],
                             xr[:, j * CHF:(j + 1) * CHF], AF.Exp)
        nc.vector.reduce_sum(s12[:, j * NW:(j + 1) * NW],
                             xr3[:, j * NW:(j + 1) * NW, :], axis=AX.X)

    # sc[n, (t,b)] = max(x_sum/8, 0) via transposing matmuls (lhsT = xt slice)
    sc_ps = psS.tile([P, NTILE * BPC], F32, tag="scp")
    for t in range(NTILE):
        nc.tensor.matmul(sc_ps[:, t * BPC:(t + 1) * BPC],
                         lhsT=xt[:, t * P:(t + 1) * P], rhs=bones[:],
                         start=True, stop=True, skip_group_check=True)
    sc_sb = smallp.tile([P, NTILE * BPC], F32, tag="scsb")
    nc.vector.tensor_scalar(sc_sb[:], sc_ps[:], 0.0, None, OP.max)

    # xws = s12 / Z (bf16 for the PE);  w1 = rowsum(xws)
    Z = smallp.tile([P, 1], F32, tag="Z")
    nc.vector.reduce_sum(Z[:], s12[:], axis=AX.X)
    rZ = smallp.tile([P, 1], F32, tag="rZ")
    nc.vector.reciprocal(rZ[:], Z[:])
    xws = sp.tile([P, N], BF16)
    nc.vector.tensor_scalar(xws[:], s12[:], rZ[:], None, OP.mult)
    w1f = smallp.tile([P, 1], F32, tag="w1f")
    nc.vector.reduce_sum(w1f[:], xws[:], axis=AX.X)
    w1 = smallp.tile([P, 1], BF16, tag="w1")
    nc.vector.tensor_copy(w1[:], w1f[:])

    # A_l denominators: sigma = gram @ 1 via sigma[n] = sum_c xws[c,n] w1[c]
    sig_ps = psS.tile([P, BPC * NTILE], F32, tag="colp")
    for b in range(BPC):
        for t in range(NTILE):
            col = b * NTILE + t
            nc.tensor.matmul(sig_ps[:, col:col + 1],
                             lhsT=xws[C * b:C * (b + 1), t * P:(t + 1) * P],
                             rhs=w1[C * b:C * (b + 1), :], start=True, stop=True,
                             skip_group_check=True)
    den = smallp.tile([P, BPC * NTILE], F32, tag="den")
    nc.vector.tensor_scalar(den[:], sig_ps[:], ISC, float(N), OP.mult, OP.add)
    rl = smallp.tile([P, BPC * NTILE], F32, tag="rl")
    nc.vector.reciprocal(rl[:], den[:])
    rl8 = smallp.tile([P, BPC * NTILE], F32, tag="rl8")
    nc.vector.tensor_scalar(rl8[:], rl[:], ISC, None, OP.mult)

    # ---------------- outputs (per-tile DMAs, alternating rings) ----------
    for b in range(BPC):
        ape = stagep.tile([P, NTILE, N], F32, tag="ape")
        apz = smallp.tile([P, NTILE], F32, tag="apz%d" % b)
        for t in range(NTILE):
            nc.scalar.activation(ape[:, t, :], sup_ps[:, t, :], AF.Exp,
                                 scale=sc_sb[:, t * BPC + b:t * BPC + b + 1],
                                 accum_out=apz[:, t:t + 1])
        apr = smallp.tile([P, NTILE], F32, tag="apr%d" % b)
        nc.vector.reciprocal(apr[:], apz[:])
        for t in range(NTILE):
            nc.vector.tensor_scalar(ape[:, t, :], ape[:, t, :],
                                    apr[:, t:t + 1], None, OP.mult)
            eng = nc.sync if t % 2 == 0 else nc.scalar
            eng.dma_start(out_v[b, 0, :, t, :], ape[:, t, :])

        ale = stagep.tile([P, NTILE, N], F32, tag="ale")
        for t in range(NTILE):
            col = b * NTILE + t
            g_ps = psB.tile([P, N], F32, tag="gram")
            nc.tensor.matmul(g_ps[:],
                             lhsT=xws[C * b:C * (b + 1), t * P:(t + 1) * P],
                             rhs=xws[C * b:C * (b + 1), :], start=True, stop=True)
            nc.scalar.activation(ale[:, t, :], g_ps[:], AF.Identity,
                                 bias=rl[:, col:col + 1],
                                 scale=rl8[:, col:col + 1])
            eng = nc.sync if t % 2 == 1 else nc.scalar
            eng.dma_start(out_v[b, 1, :, t, :], ale[:, t, :])


def build_nc():
    nc = bacc.Bacc("TRN2", target_bir_lowering=False, debug=False,
                   num_devices=NCORES)
    x_d = nc.dram_tensor("x", [BPC, C, N, T], F32, kind="ExternalInput")
    mem_d = nc.dram_tensor("memory", [C, N], F32, kind="ExternalInput")
    eye_d = nc.dram_tensor("eye", [P, P], F32, kind="ExternalInput")
    out_d = nc.dram_tensor("out", [BPC, 2, N, N], F32, kind="ExternalOutput")
    from contextlib import ExitStack
    with tile.TileContext(nc) as tc:
        with ExitStack() as ctx:
            _body(ctx, nc, tc, x_d, mem_d, eye_d, out_d)
    nc.compile()
    return nc


_NC = None


def _get_nc():
    global _NC
    if _NC is None:
        _NC = build_nc()
    return _NC


def run(x, memory, trace=False):
    nc = _get_nc()
    x = np.ascontiguousarray(np.asarray(x, dtype=np.float32))
    memory = np.ascontiguousarray(np.asarray(memory, dtype=np.float32))
    eye = np.eye(P, dtype=np.float32)
    in_maps = [
        {"x": np.ascontiguousarray(x[i * BPC:(i + 1) * BPC]),
         "memory": memory, "eye": eye}
        for i in range(NCORES)
    ]
    res = run_bass_kernel_spmd(nc, in_maps, core_ids=list(range(NCORES)),
                               trace=trace)
    full = np.concatenate([r["out"] for r in res.results], axis=0)
    return (full[:, 0], full[:, 1]), res


def kernel(x, memory):
    (a_p, a_l), _ = run(x, memory, trace=False)
    return a_p, a_l
